# revision 2
# baseline (speedup 1.0000x reference)
"""Quantized linear (dynamic per-tensor int8) on 8 TRN2 NeuronCores.

Reference semantics:
    x_q = round(x / s_x), s_x = max|x|/127   (per-tensor, round-half-even)
    w_q = round(w / s_w), s_w = max|w|/127
    out = (x_q @ w_q.T) * (s_x * s_w) + bias

Distribution: data-parallel over M (8 shards of 1024 rows), weight
replicated.  Each core scans a disjoint 1/8 of x (its own shard) and of w
for the local absmax; a 2-element AllReduce(max) collective produces the
global scales.  Quantized values are exact small integers, held in bf16
(ints <= 127 are exact in bf16), so the TensorE bf16 matmul with fp32 PSUM
accumulation reproduces the int8 GEMM exactly (sums stay far below 2^24).

Rounding uses the fp32 magic-number trick: RNE(round(v)) == (v + 1.5*2^23)
- 1.5*2^23 for |v| <= 2^22, matching jnp.round (half-to-even).

Host-side work is layout only: inputs are passed transposed (K-major) so
both matmul operands land in SBUF with K on the partition axis without any
on-device transposes; the output is computed as out^T (N on partitions) so
the bias add is a per-partition ScalarE bias, and the host transposes back.
"""

import numpy as np

from concourse import bacc, bass_isa
import concourse.bass_utils as bass_utils
import concourse.mybir as mybir
import concourse.tile as tile

P = 128
M, K, N = 8192, 4096, 4096
NCORES = 8
MLOC = M // NCORES  # 1024 rows of x per core
WS = N // NCORES  # 512 columns of wT scanned per core for absmax
MAGIC = float(np.float32(1.5 * 2**23))
MFREE = 512  # moving free dim per matmul (one fp32 PSUM bank)
NSTRIP = 128  # n-columns of w quantized per strip

F32 = mybir.dt.float32
BF16 = mybir.dt.bfloat16
AX = mybir.AxisListType
ALU = mybir.AluOpType
ACTF = mybir.ActivationFunctionType


def build_body(tc, xT, wT, wscanT, bias, outT, *, n_cores, mfree, nstrip):
    nc = tc.nc
    k, m_loc = xT.shape
    n = wT.shape[1]
    ws = wscanT.shape[1]
    kt_n = k // P
    assert k % P == 0 and n % nstrip == 0 and nstrip % P == 0 and m_loc % mfree == 0

    with (
        tc.tile_pool(name="const", bufs=1) as const,
        tc.tile_pool(name="stats", bufs=1) as stats,
        tc.tile_pool(name="xf", bufs=3) as xf_pool,
        tc.tile_pool(name="xq", bufs=1) as xq_pool,
        tc.tile_pool(name="wf", bufs=2) as wf_pool,
        tc.tile_pool(name="wq", bufs=2) as wq_pool,
        tc.tile_pool(name="ob", bufs=4) as ob_pool,
        tc.tile_pool(name="ps", bufs=4, space="PSUM") as ps_pool,
        tc.tile_pool(name="dram", bufs=1, space="DRAM") as dram,
    ):
        # ---- bias, laid out bias[j*128+p] -> bias_sb[p, j] --------------
        bias_sb = const.tile([P, n // P], F32)
        nc.sync.dma_start(bias_sb[:], bias.rearrange("(nt p) -> p nt", p=P))

        # ---- phase A: local absmax of the x shard and the w scan slice --
        xmax_cols = stats.tile([P, kt_n], F32)
        wmax_cols = stats.tile([P, kt_n], F32)
        for i in range(kt_n):
            t = xf_pool.tile([P, m_loc], F32, tag="xf")
            nc.sync.dma_start(t[:], xT[i * P : (i + 1) * P, :])
            nc.vector.tensor_reduce(
                xmax_cols[:, i : i + 1], t[:], axis=AX.X, op=ALU.max,
                apply_absolute_value=True,
            )
            tw = xf_pool.tile([P, ws], F32, tag="wscan")
            nc.sync.dma_start(tw[:], wscanT[i * P : (i + 1) * P, :])
            nc.vector.tensor_reduce(
                wmax_cols[:, i : i + 1], tw[:], axis=AX.X, op=ALU.max,
                apply_absolute_value=True,
            )

        lmax = stats.tile([P, 2], F32)
        nc.vector.tensor_reduce(lmax[:, 0:1], xmax_cols[:], axis=AX.X, op=ALU.max)
        nc.vector.tensor_reduce(lmax[:, 1:2], wmax_cols[:], axis=AX.X, op=ALU.max)
        gmax_p = stats.tile([P, 2], F32)
        nc.gpsimd.partition_all_reduce(
            gmax_p[:], lmax[:], channels=P, reduce_op=bass_isa.ReduceOp.max
        )

        # ---- phase B: AllReduce(max) of (xmax, wmax) across cores -------
        cc_in = dram.tile([1, 2], F32)
        cc_out = dram.tile([1, 2], F32)
        nc.sync.dma_start(cc_in[:], gmax_p[0:1, :])
        nc.gpsimd.collective_compute(
            "AllReduce",
            ALU.max,
            replica_groups=[list(range(n_cores))],
            ins=[cc_in.opt()],
            outs=[cc_out.opt()],
        )
        gmax = stats.tile([1, 2], F32)
        nc.sync.dma_start(gmax[:], cc_out[:])

        # scales: inv = 127/gmax for quantize; out_scale ~= (gx/127)*(gw/127)
        # (DVE has no divide ALU op; reciprocal is the HW iterative divide)
        sc3 = stats.tile([1, 3], F32)
        rec = stats.tile([1, 2], F32)
        nc.vector.reciprocal(rec[:], gmax[:])
        nc.vector.tensor_scalar(sc3[:, 0:2], rec[:], 127.0, None, op0=ALU.mult)
        s2 = stats.tile([1, 2], F32)
        nc.vector.tensor_scalar(
            s2[:], gmax[:], float(np.float32(1.0 / 127.0)), None, op0=ALU.mult
        )
        nc.vector.tensor_tensor(sc3[:, 2:3], s2[:, 0:1], s2[:, 1:2], op=ALU.mult)
        scb = const.tile([P, 3], F32)
        nc.gpsimd.partition_broadcast(scb[:], sc3[:])
        inv_sx = scb[:, 0:1]
        inv_sw = scb[:, 1:2]
        out_sc = scb[:, 2:3]

        # ---- phase C: quantize x shard -> resident bf16 [K,M] tiles -----
        xq = xq_pool.tile([P, kt_n, m_loc], BF16)
        for i in range(kt_n):
            xf = xf_pool.tile([P, m_loc], F32, tag="xf")
            nc.sync.dma_start(xf[:], xT[i * P : (i + 1) * P, :])
            nc.vector.tensor_scalar(
                xf[:], xf[:], inv_sx, MAGIC, op0=ALU.mult, op1=ALU.add
            )
            nc.vector.tensor_scalar(
                xq[:, i, :], xf[:], MAGIC, None, op0=ALU.subtract
            )

        # ---- phase D: stream w strips, quantize, matmul, evict ----------
        wT3 = wT.rearrange("(kt p) n -> p kt n", p=P)
        for s in range(n // nstrip):
            wf = wf_pool.tile([P, kt_n, nstrip], F32, tag="wf")
            nc.sync.dma_start(wf[:], wT3[:, :, s * nstrip : (s + 1) * nstrip])
            nc.vector.tensor_scalar(
                wf[:], wf[:], inv_sw, MAGIC, op0=ALU.mult, op1=ALU.add
            )
            wq = wq_pool.tile([P, kt_n, nstrip], BF16, tag="wq")
            nc.vector.tensor_scalar(wq[:], wf[:], MAGIC, None, op0=ALU.subtract)
            for nt in range(nstrip // P):
                gn = s * nstrip + nt * P  # global n of this out^T row-tile
                for mh in range(m_loc // mfree):
                    ps = ps_pool.tile([P, mfree], F32)
                    for kt in range(kt_n):
                        nc.tensor.matmul(
                            ps[:],
                            wq[:, kt, nt * P : (nt + 1) * P],
                            xq[:, kt, mh * mfree : (mh + 1) * mfree],
                            start=(kt == 0),
                            stop=(kt == kt_n - 1),
                        )
                    ob = ob_pool.tile([P, mfree], F32, tag="ob")
                    nc.scalar.activation(
                        ob[:], ps[:], ACTF.Identity,
                        bias=bias_sb[:, gn // P : gn // P + 1], scale=out_sc,
                    )
                    nc.sync.dma_start(
                        outT[gn : gn + P, mh * mfree : (mh + 1) * mfree], ob[:]
                    )


def build_nc(m_loc=MLOC, k=K, n=N, ws=WS, n_cores=NCORES, mfree=MFREE, nstrip=NSTRIP):
    nc = bacc.Bacc("TRN2", target_bir_lowering=False, debug=False,
                   num_devices=n_cores)
    xT = nc.dram_tensor("xT", [k, m_loc], F32, kind="ExternalInput").ap()
    wT = nc.dram_tensor("wT", [k, n], F32, kind="ExternalInput").ap()
    wscanT = nc.dram_tensor("wscanT", [k, ws], F32, kind="ExternalInput").ap()
    bias = nc.dram_tensor("bias", [n], F32, kind="ExternalInput").ap()
    outT = nc.dram_tensor("outT", [n, m_loc], F32, kind="ExternalOutput").ap()
    with tile.TileContext(nc) as tc:
        build_body(tc, xT, wT, wscanT, bias, outT,
                   n_cores=n_cores, mfree=mfree, nstrip=nstrip)
    nc.compile()
    return nc


def make_in_maps(x, weight, bias, n_cores=NCORES):
    m_loc = x.shape[0] // n_cores
    ws = weight.shape[0] // n_cores
    wT = np.ascontiguousarray(weight.T)
    bias = np.ascontiguousarray(bias, dtype=np.float32)
    maps = []
    for c in range(n_cores):
        maps.append({
            "xT": np.ascontiguousarray(x[c * m_loc : (c + 1) * m_loc].T),
            "wT": wT,
            "wscanT": np.ascontiguousarray(weight[c * ws : (c + 1) * ws].T),
            "bias": bias,
        })
    return maps


_NC_CACHE = {}
LAST_RUN = None


def kernel(x, weight, bias, _trace=False):
    global LAST_RUN
    x = np.ascontiguousarray(np.asarray(x), dtype=np.float32)
    weight = np.ascontiguousarray(np.asarray(weight), dtype=np.float32)
    bias = np.asarray(bias, dtype=np.float32)
    if "full" not in _NC_CACHE:
        _NC_CACHE["full"] = build_nc()
    nc = _NC_CACHE["full"]
    in_maps = make_in_maps(x, weight, bias)
    res = bass_utils.run_bass_kernel_spmd(
        nc, in_maps, core_ids=list(range(NCORES)), trace=_trace
    )
    LAST_RUN = res
    out = np.empty((M, N), np.float32)
    for c in range(NCORES):
        out[c * MLOC : (c + 1) * MLOC, :] = res.results[c]["outT"].T
    return out
